# revision 11
# baseline (speedup 1.0000x reference)
"""Distributed kNN retrieval kernel for Trainium2 (8 NeuronCores).

Computes, for query batch B=256 against three memory banks of N=131072 rows
(D=512): combined = (0.4*cos(q,Mq) + 0.4*cos(q,Mr) + 0.2*cos(q,Mt)) * strength,
masked below 0.3 to -1.0, then top-5 values + indices per query row
(ties broken by the lowest index, matching jax.lax.top_k).

Sharding: memory banks are split along N across the 8 cores. Each core:
  1. normalizes the query rows (f32), transposes q-hat via the PE,
  2. per 128-row memory tile: computes per-bank row norms on the Scalar
     engine (Square activation with free-axis accumulate), folds
     weight*strength/(norm+eps) into a single per-row scale, and combines the
     three banks into ONE effective memory matrix E on the Vector engine,
  3. DMA-transposes E (bf16) into matmul layout and runs q-hat @ E^T on the
     Tensor engine with f32 PSUM accumulation,
  4. applies relu(S - 0.3) into a [128, 16384] score row buffer, and extracts
     the top-8 values + indices per row with the DVE max/max_index ops
     (stable, ascending-index tie-break).
Host glue then gathers the 8*8 candidates per row and reduces to the global
top-5 (value desc, index asc) — the standard distributed-kNN merge.

Memory banks are fed to the device in bf16 (the device computes cosine
similarity of the bf16-quantized memories; scores only gate a 0.3 threshold
with >0.15 margin at bf16 precision).
"""

import sys

if "/opt/trn_rl_repo" not in sys.path:
    sys.path.insert(0, "/opt/trn_rl_repo")

import numpy as np

B = 256
D = 512
N_CORES = 8
CH = 512          # matmul moving free dim (n-chunk)
TILE = 128        # memory rows per tile
K_OUT = 5
THRESH = 0.3
EPS = 1e-8
WEIGHTS = (0.4, 0.4, 0.2)

_cache = {}


def _build(ns):
    """Build the per-core Bass program for a shard of ns memory rows."""
    import concourse.bass as bass
    import concourse.mybir as mybir
    from concourse.tile import TileContext
    from concourse.masks import make_identity
    from contextlib import ExitStack

    f32 = mybir.dt.float32
    bf16 = mybir.dt.bfloat16
    u32 = mybir.dt.uint32
    Act = mybir.ActivationFunctionType
    Op = mybir.AluOpType

    n_tiles = ns // TILE
    n_chunks = ns // CH
    tiles_per_chunk = CH // TILE

    nc = bass.Bass(trn_type="TRN2")

    q_d = nc.dram_tensor("q", [B, D], f32, kind="ExternalInput")
    mq_d = nc.dram_tensor("mq", [ns, D], bf16, kind="ExternalInput")
    mr_d = nc.dram_tensor("mr", [ns, D], bf16, kind="ExternalInput")
    mt_d = nc.dram_tensor("mt", [ns, D], bf16, kind="ExternalInput")
    st_d = nc.dram_tensor("st", [TILE, n_tiles], f32, kind="ExternalInput")
    vals_d = nc.dram_tensor("vals8", [B, 8], f32, kind="ExternalOutput")
    idx_d = nc.dram_tensor("idx8", [B, 8], u32, kind="ExternalOutput")

    q_ap = q_d.ap()
    banks = [mq_d.ap(), mr_d.ap(), mt_d.ap()]
    st_ap = st_d.ap()
    vals_ap = vals_d.ap()
    idx_ap = idx_d.ap()

    with TileContext(nc) as tc, ExitStack() as ctx:
        consts = ctx.enter_context(tc.tile_pool(name="consts", bufs=1))
        qpool = ctx.enter_context(tc.tile_pool(name="qpool", bufs=2))
        mpool = ctx.enter_context(tc.tile_pool(name="mpool", bufs=4))
        epool = ctx.enter_context(tc.tile_pool(name="epool", bufs=3))
        etpool = ctx.enter_context(tc.tile_pool(name="etpool", bufs=2))
        small = ctx.enter_context(tc.tile_pool(name="small", bufs=4))
        rowpool = ctx.enter_context(tc.tile_pool(name="rows", bufs=1))
        psum_s = ctx.enter_context(tc.tile_pool(name="psum_s", bufs=3, space="PSUM"))
        psum_q = ctx.enter_context(tc.tile_pool(name="psum_q", bufs=2, space="PSUM"))

        identity = consts.tile([128, 128], f32)
        make_identity(nc, identity)

        st_sb = consts.tile([TILE, n_tiles], f32)
        nc.sync.dma_start(st_sb, st_ap)

        neg_thresh = consts.tile([128, 1], f32)
        nc.vector.memset(neg_thresh, -THRESH)

        # ---- Query prep: q_hat = q / (||q|| + eps), PE-transposed to
        # qT[d_in_block, half, kblk, b] (bf16) for use as matmul lhsT.
        qT = consts.tile([128, 2, 4, 128], bf16)
        for half in range(2):
            qtile = qpool.tile([128, D], f32, tag="qtile")
            nc.sync.dma_start(qtile, q_ap[half * 128:(half + 1) * 128, :])
            qsq = qpool.tile([128, D], f32, tag="qsq")
            ssq = small.tile([128, 1], f32, tag="ssq")
            nc.scalar.activation(qsq, qtile, Act.Square, accum_out=ssq)
            qnrm = small.tile([128, 1], f32, tag="qnrm")
            nc.scalar.activation(qnrm, ssq, Act.Sqrt)
            qne = small.tile([128, 1], f32, tag="qne")
            nc.vector.tensor_scalar_add(qne, qnrm, EPS)
            qfac = small.tile([128, 1], f32, tag="qfac")
            nc.vector.reciprocal(qfac, qne)
            qhat = qpool.tile([128, D], f32, tag="qhat")
            nc.vector.tensor_scalar_mul(qhat, qtile, qfac)
            for kb in range(4):
                pt = psum_q.tile([128, 128], f32, tag="qtr")
                nc.tensor.transpose(pt, qhat[:, kb * 128:(kb + 1) * 128], identity)
                nc.scalar.activation(qT[:, half, kb, :], pt, Act.Copy)

        row0 = rowpool.tile([128, ns], bf16, tag="row0")
        row1 = rowpool.tile([128, ns], bf16, tag="row1")
        row_bufs = [row0, row1]

        # ---- Main loop over n-chunks of 512 memory rows.
        for c in range(n_chunks):
            et = etpool.tile([128, 4, CH], bf16, tag="et")
            for j in range(tiles_per_chunk):
                t = c * tiles_per_chunk + j
                m_tiles = []
                for bi in range(3):
                    mtile = mpool.tile([128, D], bf16, tag=f"m{bi}")
                    nc.sync.dma_start(mtile, banks[bi][t * TILE:(t + 1) * TILE, :])
                    m_tiles.append(mtile)

                # Row sum-of-squares per bank, scaled by 1/w^2 so that
                # 1/(sqrt(ss') + eps) = w/(||m|| + w*eps): the bank weight is
                # folded into the normalization for free.
                ss3 = small.tile([128, 3], f32, tag="ss3")
                for bi, w in enumerate(WEIGHTS):
                    sq = epool.tile([128, D], bf16, tag="sq")
                    nc.scalar.activation(
                        sq, m_tiles[bi], Act.Square,
                        scale=float(1.0 / w), accum_out=ss3[:, bi:bi + 1],
                    )
                nrm3 = small.tile([128, 3], f32, tag="nrm3")
                nc.scalar.activation(nrm3, ss3, Act.Sqrt)
                ne3 = small.tile([128, 3], f32, tag="ne3")
                nc.vector.tensor_scalar_add(ne3, nrm3, EPS)
                g3 = small.tile([128, 3], f32, tag="g3")
                nc.vector.reciprocal(g3, ne3)
                a3 = small.tile([128, 3], f32, tag="a3")
                nc.vector.tensor_scalar_mul(a3, g3, st_sb[:, t:t + 1])

                # E = sum_banks a_bank * M_bank   (per-partition row scales)
                e32 = epool.tile([128, D], f32, tag="e32")
                nc.vector.tensor_scalar_mul(e32, m_tiles[0], a3[:, 0:1])
                nc.vector.scalar_tensor_tensor(
                    e32, m_tiles[1], a3[:, 1:2], e32, op0=Op.mult, op1=Op.add)
                ebf = epool.tile([128, D], bf16, tag="ebf")
                nc.vector.scalar_tensor_tensor(
                    ebf, m_tiles[2], a3[:, 2:3], e32, op0=Op.mult, op1=Op.add)

                # Blocked transpose via DMA xbar: et[p, kb, n] = E[n, kb*128+p]
                nc.sync.dma_start(
                    et[:, :, j * TILE:(j + 1) * TILE], ebf, transpose=True)

            for half in range(2):
                ps = psum_s.tile([128, CH], f32, tag="S")
                for kb in range(4):
                    nc.tensor.matmul(
                        ps, qT[:, half, kb, :], et[:, kb, :],
                        start=(kb == 0), stop=(kb == 3),
                    )
                # row_buf = relu(S - 0.3): 0 for masked entries, shifted
                # score otherwise (order-preserving; ties at 0 resolve by
                # ascending index in max_index, matching top_k).
                nc.scalar.activation(
                    row_bufs[half][:, c * CH:(c + 1) * CH], ps,
                    Act.Relu, bias=neg_thresh[:],
                )

        # ---- Local top-8 per query row.
        for half in range(2):
            top8 = small.tile([128, 8], bf16, tag="top8")
            nc.vector.max(out=top8, in_=row_bufs[half])
            i8 = small.tile([128, 8], u32, tag="i8")
            nc.vector.max_index(out=i8, in_max=top8, in_values=row_bufs[half])
            # vals = v > 0 ? v + 0.3 : -1.0
            vplus = small.tile([128, 8], f32, tag="vplus")
            nc.vector.tensor_scalar(vplus, top8, float(THRESH), None, op0=Op.add)
            msk = small.tile([128, 8], f32, tag="msk")
            nc.vector.tensor_scalar(msk, top8, 0.0, None, op0=Op.is_gt)
            vsel = small.tile([128, 8], f32, tag="vsel")
            nc.vector.tensor_tensor(vsel, msk, vplus, op=Op.mult)
            mm1 = small.tile([128, 8], f32, tag="mm1")
            nc.vector.tensor_scalar(mm1, msk, 1.0, None, op0=Op.subtract)
            vout = small.tile([128, 8], f32, tag="vout")
            nc.vector.tensor_tensor(vout, vsel, mm1, op=Op.add)
            nc.sync.dma_start(vals_ap[half * 128:(half + 1) * 128, :], vout)
            nc.sync.dma_start(idx_ap[half * 128:(half + 1) * 128, :], i8)

    _split_tsp_waits(nc, mybir)
    return nc


def _split_tsp_waits(nc, mybir):
    """This walrus build rejects ANY instruction carrying more than one
    sync-wait command in its encoding (TensorScalarPtr at birverifier;
    LdWeights/Matmult/DMACopy at codegen's setupSyncWait — verified
    empirically: trimming every instruction to one wait compiles). Hoist
    excess waits onto same-engine NoOps inserted just before — engines
    execute their stream in order, so gating the NoOp gates the op. The
    emitted stream order is a valid topological order of Tile's dependency
    graph, so blocking the issuing sequencer on a hoisted wait cannot
    deadlock."""
    skip = {"NoOp"}
    fn = nc.m.functions[0]
    for blk in fn.blocks:
        insts = list(blk.instructions)
        new_insts = []
        changed = False
        for ins in insts:
            si = ins.sync_info
            waits = list(si.on_wait) if si is not None and si.on_wait else []
            if ins.opcode not in skip and len(waits) > 1:
                for wi, w in enumerate(waits[:-1]):
                    new_insts.append(mybir.InstNoOp(
                        name=f"{ins.name}-wn{wi}",
                        engine=ins.engine,
                        sync_info=mybir.SyncInfo(on_wait=[w], on_update=[]),
                    ))
                ins.sync_info = mybir.SyncInfo(
                    on_wait=waits[-1:],
                    on_update=list(si.on_update) if si.on_update else [],
                )
                changed = True
            new_insts.append(ins)
        if changed:
            blk.instructions = new_insts


def _get_program(ns):
    if ns not in _cache:
        _cache[ns] = _build(ns)
    return _cache[ns]


def make_in_maps(query, mem_questions, mem_responses, mem_traces, mem_strengths):
    """Host-side sharding + bf16 cast. Returns per-core input dicts."""
    import ml_dtypes

    q = np.ascontiguousarray(np.asarray(query, dtype=np.float32))
    s = np.asarray(mem_strengths, dtype=np.float32)
    banks = [
        np.asarray(x, dtype=np.float32).astype(ml_dtypes.bfloat16)
        for x in (mem_questions, mem_responses, mem_traces)
    ]
    n = banks[0].shape[0]
    ns = n // N_CORES
    in_maps = []
    for c in range(N_CORES):
        sl = slice(c * ns, (c + 1) * ns)
        st_packed = np.ascontiguousarray(s[sl].reshape(ns // TILE, TILE).T)
        in_maps.append({
            "q": q,
            "mq": np.ascontiguousarray(banks[0][sl]),
            "mr": np.ascontiguousarray(banks[1][sl]),
            "mt": np.ascontiguousarray(banks[2][sl]),
            "st": st_packed,
        })
    return in_maps, ns


def merge_candidates(per_core, ns, k):
    """Gather 8 candidates per core per row, reduce to global top-k
    (value desc, global index asc) — matches jax.lax.top_k tie-breaking."""
    cand_vals = np.concatenate([r["vals8"] for r in per_core], axis=1)
    cand_idx = np.concatenate(
        [r["idx8"].astype(np.int64) + c * ns for c, r in enumerate(per_core)],
        axis=1,
    )
    order1 = np.argsort(cand_idx, axis=1, kind="stable")
    v1 = np.take_along_axis(cand_vals, order1, axis=1)
    i1 = np.take_along_axis(cand_idx, order1, axis=1)
    order2 = np.argsort(-v1, axis=1, kind="stable")
    vals = np.take_along_axis(v1, order2, axis=1)[:, :k]
    idx = np.take_along_axis(i1, order2, axis=1)[:, :k]
    return vals.astype(np.float32), idx.astype(np.int32)


def _install_ntff_shim():
    """Register the axon NTFF profile hook (the agent image lacks
    antenv.axon_hooks; recreate it per the documented ctypes C ABI)."""
    import sys as _sys
    import types
    import ctypes
    import contextlib

    if "antenv.axon_hooks" in _sys.modules:
        return
    so_path = "/opt/axon/libaxon_pjrt.so"
    lib = ctypes.CDLL(so_path)
    if not hasattr(lib, "axon_start_nrt_profile"):
        return
    lib.axon_start_nrt_profile.argtypes = [
        ctypes.POINTER(ctypes.c_int64), ctypes.c_size_t]
    lib.axon_start_nrt_profile.restype = ctypes.c_int64
    lib.axon_stop_nrt_profile.argtypes = [ctypes.c_char_p]
    lib.axon_stop_nrt_profile.restype = ctypes.c_int64

    @contextlib.contextmanager
    def _hook(output_dir, device_ids):
        import jax
        jax.devices()
        if device_ids:
            ids = (ctypes.c_int64 * len(device_ids))(*device_ids)
            rc = lib.axon_start_nrt_profile(ids, len(device_ids))
        else:
            rc = lib.axon_start_nrt_profile(None, 0)
        if rc != 0:
            raise RuntimeError(f"axon_start_nrt_profile rc={rc}")
        try:
            yield
        finally:
            n = lib.axon_stop_nrt_profile(str(output_dir).encode())
            print(f"ntff profile: {n} file(s) written to {output_dir}",
                  file=_sys.stderr)

    mod = types.ModuleType("antenv.axon_hooks")
    mod._hook = _hook
    mod.get_axon_ntff_profile_hook = lambda: _hook
    mod.set_axon_ntff_profile_hook = lambda h: None
    _sys.modules["antenv.axon_hooks"] = mod


def kernel(query, mem_questions, mem_responses, mem_traces, mem_strengths,
           top_k, _trace=False, _results_box=None):
    from concourse import bass_utils

    if _trace:
        _install_ntff_shim()

    k = int(top_k)
    in_maps, ns = make_in_maps(
        query, mem_questions, mem_responses, mem_traces, mem_strengths)
    nc = _get_program(ns)
    res = bass_utils.run_bass_kernel_spmd(
        nc, in_maps, core_ids=list(range(N_CORES)), trace=_trace)
    if _results_box is not None:
        _results_box.append(res)
    return merge_candidates(res.results, ns, k)


# revision 18
# speedup vs baseline: 1.8040x; 1.8040x over previous
"""Distributed kNN retrieval kernel for Trainium2 (8 NeuronCores).

Computes, for query batch B=256 against three memory banks of N=131072 rows
(D=512): combined = (0.4*cos(q,Mq) + 0.4*cos(q,Mr) + 0.2*cos(q,Mt)) * strength,
masked below 0.3 to -1.0, then top-5 values + indices per query row
(ties broken by the lowest index, matching jax.lax.top_k).

Sharding: memory banks are split along N across the 8 cores. Each core:
  1. normalizes the query rows (f32), transposes q-hat via the PE,
  2. per 128-row memory tile: computes per-bank row norms on the Scalar
     engine (Square activation with free-axis accumulate), folds
     weight*strength/(norm+eps) into a single per-row scale, and combines the
     three banks into ONE effective memory matrix E on the Vector engine,
  3. DMA-transposes E (bf16) into matmul layout and runs q-hat @ E^T on the
     Tensor engine with f32 PSUM accumulation,
  4. applies relu(S - 0.3) into a [128, 16384] score row buffer, and extracts
     the top-8 values + indices per row with the DVE max/max_index ops
     (stable, ascending-index tie-break).
Host glue then gathers the 8*8 candidates per row and reduces to the global
top-5 (value desc, index asc) — the standard distributed-kNN merge.

Memory banks are fed to the device in bf16 (the device computes cosine
similarity of the bf16-quantized memories; scores only gate a 0.3 threshold
with >0.15 margin at bf16 precision).
"""

import sys

if "/opt/trn_rl_repo" not in sys.path:
    sys.path.insert(0, "/opt/trn_rl_repo")

import numpy as np

B = 256
D = 512
N_CORES = 8
CH = 512          # matmul moving free dim (n-chunk)
TILE = 128        # memory rows per tile
K_OUT = 5
THRESH = 0.3
EPS = 1e-8
WEIGHTS = (0.4, 0.4, 0.2)

_cache = {}


def _build(ns, split_waits=True):
    """Build the per-core Bass program for a shard of ns memory rows."""
    import concourse.bass as bass
    import concourse.mybir as mybir
    from concourse.tile import TileContext
    from concourse.masks import make_identity
    from contextlib import ExitStack

    f32 = mybir.dt.float32
    bf16 = mybir.dt.bfloat16
    u32 = mybir.dt.uint32
    Act = mybir.ActivationFunctionType
    Op = mybir.AluOpType

    n_tiles = ns // TILE
    n_chunks = ns // CH
    tiles_per_chunk = CH // TILE

    nc = bass.Bass(trn_type="TRN2")

    q_d = nc.dram_tensor("q", [B, D], f32, kind="ExternalInput")
    mq_d = nc.dram_tensor("mq", [ns, D], bf16, kind="ExternalInput")
    mr_d = nc.dram_tensor("mr", [ns, D], bf16, kind="ExternalInput")
    mt_d = nc.dram_tensor("mt", [ns, D], bf16, kind="ExternalInput")
    st_d = nc.dram_tensor("st", [TILE, n_tiles], f32, kind="ExternalInput")
    vals_d = nc.dram_tensor("vals8", [B, 8], f32, kind="ExternalOutput")
    idx_d = nc.dram_tensor("idx8", [B, 8], u32, kind="ExternalOutput")

    q_ap = q_d.ap()
    banks = [mq_d.ap(), mr_d.ap(), mt_d.ap()]
    st_ap = st_d.ap()
    vals_ap = vals_d.ap()
    idx_ap = idx_d.ap()

    with TileContext(nc) as tc, ExitStack() as ctx:
        consts = ctx.enter_context(tc.tile_pool(name="consts", bufs=1))
        qpool = ctx.enter_context(tc.tile_pool(name="qpool", bufs=2))
        mpool = ctx.enter_context(tc.tile_pool(name="mpool", bufs=3))
        epool = ctx.enter_context(tc.tile_pool(name="epool", bufs=3))
        etpool = ctx.enter_context(tc.tile_pool(name="etpool", bufs=3))
        small = ctx.enter_context(tc.tile_pool(name="small", bufs=4))
        rowpool = ctx.enter_context(tc.tile_pool(name="rows", bufs=1))
        psum_s = ctx.enter_context(tc.tile_pool(name="psum_s", bufs=4, space="PSUM"))
        psum_q = ctx.enter_context(tc.tile_pool(name="psum_q", bufs=2, space="PSUM"))

        identity = consts.tile([128, 128], f32)
        make_identity(nc, identity)

        st_sb = consts.tile([TILE, n_tiles], f32)
        nc.sync.dma_start(st_sb, st_ap)

        neg_thresh = consts.tile([128, 1], f32)
        nc.vector.memset(neg_thresh, -THRESH)

        # ---- Query prep: q_hat = q / (||q|| + eps), PE-transposed to
        # qT[d_in_block, half, kblk, b] (bf16) for use as matmul lhsT.
        qT = consts.tile([128, 2, 4, 128], bf16)
        for half in range(2):
            qtile = qpool.tile([128, D], f32, tag="qtile")
            nc.sync.dma_start(qtile, q_ap[half * 128:(half + 1) * 128, :])
            qsq = qpool.tile([128, D], f32, tag="qsq")
            ssq = small.tile([128, 1], f32, tag="ssq")
            nc.scalar.activation(qsq, qtile, Act.Square, accum_out=ssq)
            qnrm = small.tile([128, 1], f32, tag="qnrm")
            nc.scalar.activation(qnrm, ssq, Act.Sqrt)
            qne = small.tile([128, 1], f32, tag="qne")
            nc.vector.tensor_scalar_add(qne, qnrm, EPS)
            qfac = small.tile([128, 1], f32, tag="qfac")
            nc.vector.reciprocal(qfac, qne)
            qhat = qpool.tile([128, D], f32, tag="qhat")
            nc.vector.tensor_scalar_mul(qhat, qtile, qfac)
            for kb in range(4):
                pt = psum_q.tile([128, 128], f32, tag="qtr")
                nc.tensor.transpose(pt, qhat[:, kb * 128:(kb + 1) * 128], identity)
                nc.scalar.activation(qT[:, half, kb, :], pt, Act.Copy)

        row0 = rowpool.tile([128, ns], bf16, tag="row0")
        row1 = rowpool.tile([128, ns], bf16, tag="row1")
        row_bufs = [row0, row1]
        # Per-quarter top-8 candidates, extracted while the main loop runs so
        # only the last quarter's extraction is on the critical tail.
        qc0 = rowpool.tile([128, 32], bf16, tag="qc0")
        qc1 = rowpool.tile([128, 32], bf16, tag="qc1")
        qcand = [qc0, qc1]
        q_chunks = n_chunks // 4

        # ---- Main loop over n-chunks of 512 memory rows.
        for c in range(n_chunks):
            # One DMA per bank per chunk: [p, j, d] = bank[c*512 + j*128 + p, d]
            m_tiles = []
            for bi in range(3):
                mtile = mpool.tile([128, tiles_per_chunk, D], bf16, tag=f"m{bi}")
                src = banks[bi][c * CH:(c + 1) * CH, :].rearrange(
                    "(j p) d -> p j d", p=128)
                nc.sync.dma_start(mtile, src)
                m_tiles.append(mtile)

            # Row sum-of-squares per (tile, bank), scaled by 1/w^2 so that
            # 1/(sqrt(ss') + eps) = w/(||m|| + w*eps): the bank weight is
            # folded into the normalization for free. Batched factor math:
            # one sqrt/recip/mul per chunk. ss12 column = j*3 + bank.
            ss12 = small.tile([128, 12], f32, tag="ss12")
            for j in range(tiles_per_chunk):
                for bi, w in enumerate(WEIGHTS):
                    sq = epool.tile([128, D], bf16, tag=f"sq{bi}")
                    nc.scalar.activation(
                        sq, m_tiles[bi][:, j, :], Act.Square,
                        scale=float(1.0 / w),
                        accum_out=ss12[:, j * 3 + bi:j * 3 + bi + 1])
            nrm12 = small.tile([128, 12], f32, tag="nrm12")
            nc.scalar.activation(nrm12, ss12, Act.Sqrt)
            ne12 = small.tile([128, 12], f32, tag="ne12")
            nc.vector.tensor_scalar_add(ne12, nrm12, EPS)
            g12 = small.tile([128, 12], f32, tag="g12")
            nc.vector.reciprocal(g12, ne12)
            a12 = small.tile([128, 12], f32, tag="a12")
            nc.vector.tensor_tensor(
                a12.rearrange("p (j b) -> p j b", b=3),
                g12.rearrange("p (j b) -> p j b", b=3),
                st_sb[:, c * 4:(c + 1) * 4].to_broadcast([128, 4, 3]),
                op=Op.mult)

            # E = sum_banks a_bank * M_bank (per-partition row scales),
            # all-bf16 chain for DVE 2x/4x modes.
            ebf = etpool.tile([128, tiles_per_chunk, D], bf16, tag="ebf")
            for j in range(tiles_per_chunk):
                o = j * 3
                e1 = epool.tile([128, D], bf16, tag="e1")
                nc.vector.tensor_scalar_mul(
                    e1, m_tiles[0][:, j, :], a12[:, o:o + 1])
                e2 = epool.tile([128, D], bf16, tag="e2")
                nc.vector.scalar_tensor_tensor(
                    e2, m_tiles[1][:, j, :], a12[:, o + 1:o + 2], e1,
                    op0=Op.mult, op1=Op.add)
                nc.vector.scalar_tensor_tensor(
                    ebf[:, j, :], m_tiles[2][:, j, :], a12[:, o + 2:o + 3], e2,
                    op0=Op.mult, op1=Op.add)

            # One blocked transpose per chunk via the DMA xbar:
            # et[p, k, n] = E_tile[j=k//4][n, (k%4)*128 + p]  (k = 4j + kb)
            et = etpool.tile([128, 4 * tiles_per_chunk, TILE], bf16, tag="et")
            nc.sync.dma_start(et, ebf, transpose=True)
            et_k = et.rearrange("p (j kb) n -> p kb j n", kb=4)

            for half in range(2):
                ps = psum_s.tile([128, CH], f32, tag="S")
                for kb in range(4):
                    nc.tensor.matmul(
                        ps, qT[:, half, kb, :], et_k[:, kb, :, :],
                        start=(kb == 0), stop=(kb == 3),
                    )
                # row_buf = relu(S - 0.3): 0 for masked entries, shifted
                # score otherwise (order-preserving; ties at 0 resolve by
                # ascending index in the index-match pass, matching top_k).
                # On DVE: (S + -0.3) max 0, PSUM -> SBUF bf16.
                nc.vector.tensor_scalar(
                    row_bufs[half][:, c * CH:(c + 1) * CH], ps,
                    -THRESH, 0.0, op0=Op.add, op1=Op.max)

            if (c + 1) % q_chunks == 0:
                q = (c + 1) // q_chunks - 1
                qw = q_chunks * CH
                for half in range(2):
                    nc.vector.max(
                        out=qcand[half][:, q * 8:(q + 1) * 8],
                        in_=row_bufs[half][:, q * qw:(q + 1) * qw])

        # ---- Local top-8 per query row: top-8 of the 32 quarter candidates,
        # then one full-row pass to recover indices (stable ascending ties).
        for half in range(2):
            top8 = small.tile([128, 8], bf16, tag="top8")
            nc.vector.max(out=top8, in_=qcand[half])
            i8 = small.tile([128, 8], u32, tag="i8")
            nc.vector.max_index(out=i8, in_max=top8, in_values=row_bufs[half])
            # vals = v > 0 ? v + 0.3 : -1.0
            vplus = small.tile([128, 8], f32, tag="vplus")
            nc.vector.tensor_scalar(vplus, top8, float(THRESH), None, op0=Op.add)
            msk = small.tile([128, 8], f32, tag="msk")
            nc.vector.tensor_scalar(msk, top8, 0.0, None, op0=Op.is_gt)
            vsel = small.tile([128, 8], f32, tag="vsel")
            nc.vector.tensor_tensor(vsel, msk, vplus, op=Op.mult)
            mm1 = small.tile([128, 8], f32, tag="mm1")
            nc.vector.tensor_scalar(mm1, msk, 1.0, None, op0=Op.subtract)
            vout = small.tile([128, 8], f32, tag="vout")
            nc.vector.tensor_tensor(vout, vsel, mm1, op=Op.add)
            nc.sync.dma_start(vals_ap[half * 128:(half + 1) * 128, :], vout)
            nc.sync.dma_start(idx_ap[half * 128:(half + 1) * 128, :], i8)

    if split_waits:
        _split_tsp_waits(nc, mybir)
    return nc


def _split_tsp_waits(nc, mybir):
    """This walrus build rejects ANY instruction carrying more than one
    sync-wait command in its encoding (TensorScalarPtr at birverifier;
    LdWeights/Matmult/DMACopy at codegen's setupSyncWait — verified
    empirically: trimming every instruction to one wait compiles). Hoist
    excess waits onto same-engine NoOps inserted just before — engines
    execute their stream in order, so gating the NoOp gates the op. The
    emitted stream order is a valid topological order of Tile's dependency
    graph, so blocking the issuing sequencer on a hoisted wait cannot
    deadlock."""
    skip = {"NoOp"}
    fn = nc.m.functions[0]
    for blk in fn.blocks:
        insts = list(blk.instructions)
        new_insts = []
        changed = False
        for ins in insts:
            si = ins.sync_info
            waits = list(si.on_wait) if si is not None and si.on_wait else []
            if ins.opcode not in skip and len(waits) > 1:
                for wi, w in enumerate(waits[:-1]):
                    new_insts.append(mybir.InstNoOp(
                        name=f"{ins.name}-wn{wi}",
                        engine=ins.engine,
                        sync_info=mybir.SyncInfo(on_wait=[w], on_update=[]),
                    ))
                ins.sync_info = mybir.SyncInfo(
                    on_wait=waits[-1:],
                    on_update=list(si.on_update) if si.on_update else [],
                )
                changed = True
            new_insts.append(ins)
        if changed:
            blk.instructions = new_insts


def _get_program(ns):
    if ns not in _cache:
        _cache[ns] = _build(ns)
    return _cache[ns]


def make_in_maps(query, mem_questions, mem_responses, mem_traces, mem_strengths):
    """Host-side sharding + bf16 cast. Returns per-core input dicts."""
    import ml_dtypes

    q = np.ascontiguousarray(np.asarray(query, dtype=np.float32))
    s = np.asarray(mem_strengths, dtype=np.float32)
    banks = [
        np.asarray(x, dtype=np.float32).astype(ml_dtypes.bfloat16)
        for x in (mem_questions, mem_responses, mem_traces)
    ]
    n = banks[0].shape[0]
    ns = n // N_CORES
    in_maps = []
    for c in range(N_CORES):
        sl = slice(c * ns, (c + 1) * ns)
        st_packed = np.ascontiguousarray(s[sl].reshape(ns // TILE, TILE).T)
        in_maps.append({
            "q": q,
            "mq": np.ascontiguousarray(banks[0][sl]),
            "mr": np.ascontiguousarray(banks[1][sl]),
            "mt": np.ascontiguousarray(banks[2][sl]),
            "st": st_packed,
        })
    return in_maps, ns


def merge_candidates(per_core, ns, k):
    """Gather 8 candidates per core per row, reduce to global top-k
    (value desc, global index asc) — matches jax.lax.top_k tie-breaking."""
    cand_vals = np.concatenate([r["vals8"] for r in per_core], axis=1)
    cand_idx = np.concatenate(
        [r["idx8"].astype(np.int64) + c * ns for c, r in enumerate(per_core)],
        axis=1,
    )
    order1 = np.argsort(cand_idx, axis=1, kind="stable")
    v1 = np.take_along_axis(cand_vals, order1, axis=1)
    i1 = np.take_along_axis(cand_idx, order1, axis=1)
    order2 = np.argsort(-v1, axis=1, kind="stable")
    vals = np.take_along_axis(v1, order2, axis=1)[:, :k]
    idx = np.take_along_axis(i1, order2, axis=1)[:, :k]
    return vals.astype(np.float32), idx.astype(np.int32)


def _install_ntff_shim():
    """Register the axon NTFF profile hook (the agent image lacks
    antenv.axon_hooks; recreate it per the documented ctypes C ABI)."""
    import sys as _sys
    import types
    import ctypes
    import contextlib

    if "antenv.axon_hooks" in _sys.modules:
        return
    so_path = "/opt/axon/libaxon_pjrt.so"
    lib = ctypes.CDLL(so_path)
    if not hasattr(lib, "axon_start_nrt_profile"):
        return
    lib.axon_start_nrt_profile.argtypes = [
        ctypes.POINTER(ctypes.c_int64), ctypes.c_size_t]
    lib.axon_start_nrt_profile.restype = ctypes.c_int64
    lib.axon_stop_nrt_profile.argtypes = [ctypes.c_char_p]
    lib.axon_stop_nrt_profile.restype = ctypes.c_int64

    @contextlib.contextmanager
    def _hook(output_dir, device_ids):
        import jax
        jax.devices()
        if device_ids:
            ids = (ctypes.c_int64 * len(device_ids))(*device_ids)
            rc = lib.axon_start_nrt_profile(ids, len(device_ids))
        else:
            rc = lib.axon_start_nrt_profile(None, 0)
        if rc != 0:
            raise RuntimeError(f"axon_start_nrt_profile rc={rc}")
        try:
            yield
        finally:
            n = lib.axon_stop_nrt_profile(str(output_dir).encode())
            print(f"ntff profile: {n} file(s) written to {output_dir}",
                  file=_sys.stderr)

    mod = types.ModuleType("antenv.axon_hooks")
    mod._hook = _hook
    mod.get_axon_ntff_profile_hook = lambda: _hook
    mod.set_axon_ntff_profile_hook = lambda h: None
    _sys.modules["antenv.axon_hooks"] = mod


def kernel(query, mem_questions, mem_responses, mem_traces, mem_strengths,
           top_k, _trace=False, _results_box=None):
    from concourse import bass_utils

    if _trace:
        _install_ntff_shim()

    k = int(top_k)
    in_maps, ns = make_in_maps(
        query, mem_questions, mem_responses, mem_traces, mem_strengths)
    nc = _get_program(ns)
    res = bass_utils.run_bass_kernel_spmd(
        nc, in_maps, core_ids=list(range(N_CORES)), trace=_trace)
    if _results_box is not None:
        _results_box.append(res)
    return merge_candidates(res.results, ns, k)
